# revision 16
# baseline (speedup 1.0000x reference)
"""Multi-head attention (B=2, S=2048, D=1024, H=16, d_k=64) on 8 NeuronCores.

Sharding: 8 cores = 2 batches x 4 head-groups (4 heads each).
Core c handles batch b = c//4 and heads 4*(c%4) .. 4*(c%4)+4 (feature
slice of width F=256). Each core computes its partial output-projection
contribution [S, D]; the host sums the 4 head-group partials per batch
and adds b4 (the "all-reduce" of the row-sharded W4 projection).

Device dataflow works in a "transposed world" so every matmul operand
is in its natural PE layout (contraction on partitions), with zero
on-device transposes:
  qT = W1g @ x_q.T  [F, S]   (lhsT = W1g.T host-prepped, rhs = x_q.T host-prepped)
  kT = W2g @ x_k.T  [F, S]
  v  = x_v @ W3g.T  [S, F]   (lhsT = x_v.T, rhs = W3g.T; bias via K=1 ones matmul)
  scoresT_h = kT_h.T @ qT_h        [S_keys, 512-q window]   (K = d_k = 64)
  attnT = exp(scoresT / 8)          ACT, PSUM->SBUF bf16, no max subtraction
  pv = [v_h | ones].T @ attnT      [65, 512]; row 64 = softmax denominator
  outT_h = pv[0:64] * (1/pv[64])   per-query normalization post-PV
  partial = outT.T @ W4g.T         [S, D]

All matmuls bf16 with f32 PSUM accumulation.

Schedule (HAM-aware): the TRN2 PE clock-gate (PE_HAM) halves the PE clock
whenever recent PE activity is low, and the attention inner loop alone
cannot keep it busy (scores+PV per key tile is ~0.9us of PE vs ~1.2us of
ACT exp). So ALL independent PE work - the q/k/v projections, and the W4
output projection of completed windows - is interleaved into the
scores/PV stream by an emission-time scheduler that keeps the in-order
PE queue dense: per tick it emits <=2 scores matmuls (gated on projection
progress + attn-tile backlog), <=3 PV matmuls (gated on v-projection
progress), and ~4 filler matmuls. DMA is panel-prioritized (w1/w2,
xq/xk first 512-col panels first) so the PE starts ~7us in, and xv/v
land in time for PV to chase scores with ~1 window of lag.
"""

import numpy as np
import ml_dtypes

import concourse.bass as bass
import concourse.mybir as mybir
import concourse.tile as tile
from concourse import bacc
from concourse.bass_utils import run_bass_kernel_spmd

BF16 = ml_dtypes.bfloat16
F32 = mybir.dt.float32
BF = mybir.dt.bfloat16

B, S, D = 2, 2048, 1024
H_CORE = 4          # heads per core
DK = 64             # head dim
F = H_CORE * DK     # features per core = 256
P = 128             # partitions
KB = D // P         # k blocks in D contraction = 8
SM = S // P         # seq tiles of 128 = 16
QW = 512            # query window width
NQW = S // QW       # query windows = 4
N_CORES = 8
ATTN_BUFS = 20      # attn sbuf PAIR tiles in flight (2KB/partition each)
ATTN_CAP = 17       # emission-time backlog cap (score pairs ahead of PV)


def _build_kernel():
    nc = bacc.Bacc(
        "TRN2",
        target_bir_lowering=False,
        debug=False,
        enable_asserts=False,
        num_devices=N_CORES,
    )

    xq = nc.dram_tensor("xq_t", [D, S], BF, kind="ExternalInput").ap()
    xk = nc.dram_tensor("xk_t", [D, S], BF, kind="ExternalInput").ap()
    xv = nc.dram_tensor("xv_t", [D, S], BF, kind="ExternalInput").ap()
    w1 = nc.dram_tensor("w1t", [D, F], BF, kind="ExternalInput").ap()
    w2 = nc.dram_tensor("w2t", [D, F], BF, kind="ExternalInput").ap()
    w3 = nc.dram_tensor("w3t", [D, F], BF, kind="ExternalInput").ap()
    w4 = nc.dram_tensor("w4t", [F, D], BF, kind="ExternalInput").ap()
    b1 = nc.dram_tensor("b1c", [P, F // P], F32, kind="ExternalInput").ap()
    b2 = nc.dram_tensor("b2c", [P, F // P], F32, kind="ExternalInput").ap()
    b3 = nc.dram_tensor("b3r", [1, F], BF, kind="ExternalInput").ap()
    out = nc.dram_tensor("out", [S, D], F32, kind="ExternalOutput").ap()

    with tile.TileContext(nc) as tc:
        _body(tc, xq, xk, xv, w1, w2, w3, w4, b1, b2, b3, out)

    nc.compile()
    return nc


def _body(tc, xq, xk, xv, w1, w2, w3, w4, b1, b2, b3, out):
    nc = tc.nc
    MF = F // P  # m tiles for the F=256 feature dim = 2

    with (
        tc.tile_pool(name="wpool", bufs=1) as wpool,
        tc.tile_pool(name="xqp", bufs=2) as xq_pool,
        tc.tile_pool(name="xkp", bufs=3) as xk_pool,
        tc.tile_pool(name="xvp", bufs=2) as xv_pool,
        tc.tile_pool(name="persist", bufs=1) as persist,
        tc.tile_pool(name="attn", bufs=ATTN_BUFS) as attn_pool,
        tc.tile_pool(name="small", bufs=4) as small,
        tc.tile_pool(name="stage", bufs=3) as stage,
        tc.tile_pool(name="psum", bufs=1, space="PSUM") as psum,
    ):
        # ---- weight / bias tiles (batched: one DMA per tensor) ----
        w1_sb = wpool.tile([P, KB, F], BF, name="w1_sb", tag="w1_sb")
        w2_sb = wpool.tile([P, KB, F], BF, name="w2_sb", tag="w2_sb")
        w3_sb = wpool.tile([P, KB, F], BF, name="w3_sb", tag="w3_sb")
        w4_sb = wpool.tile([P, MF, D], BF, name="w4_sb", tag="w4_sb")
        b1_sb = wpool.tile([P, MF], F32, name="b1_sb", tag="b1_sb")
        b2_sb = wpool.tile([P, MF], F32, name="b2_sb", tag="b2_sb")
        b3_sb = wpool.tile([1, F], BF, name="b3_sb", tag="b3_sb")
        ones_row = wpool.tile([1, P], BF, name="ones_row", tag="ones_row")

        # x: one [P, KB, QW] tile per query-window column panel; xv in 2 halves
        xq_sp = {}
        xk_sp = {}
        xv_t = []
        w1_r = w1.rearrange("(kb p) f -> p kb f", p=P)
        w2_r = w2.rearrange("(kb p) f -> p kb f", p=P)
        w3_r = w3.rearrange("(kb p) f -> p kb f", p=P)
        w4_r = w4.rearrange("(m p) d -> p m d", p=P)
        xq_r = xq.rearrange("(kb p) s -> p kb s", p=P)
        xk_r = xk.rearrange("(kb p) s -> p kb s", p=P)
        xv_r = xv.rearrange("(kb p) s -> p kb s", p=P)

        # persistent activations
        qT = [persist.tile([P, S], BF, name=f"qT_{m}", tag=f"qT_{m}") for m in range(MF)]
        kT = [persist.tile([P, S], BF, name=f"kT_{m}", tag=f"kT_{m}") for m in range(MF)]
        VW = H_CORE * (DK + 1)  # 260: per head h, cols 65h..65h+63 = v_h, col 65h+64 = 1
        v_sb = [persist.tile([P, VW], BF, name=f"v_{s}", tag=f"v_{s}") for s in range(SM)]
        outT = [persist.tile([P, S], BF, name=f"outT_{m}", tag=f"outT_{m}") for m in range(MF)]

        # ---- DMA emission, in need-order (few big transfers; the Sync
        # engine issues DMAs serially at ~0.6us each, so issue count matters)
        def dma_x_sp(pool, store, x_r, which, sp, halves=1):
            t = pool.tile([P, KB, QW], BF, name=f"x{which}_{sp}", tag=f"x{which}")
            hk = KB // halves
            for j in range(halves):
                nc.sync.dma_start(
                    t[:, j * hk:(j + 1) * hk, :],
                    x_r[:, j * hk:(j + 1) * hk, sp * QW:(sp + 1) * QW],
                )
            store[sp] = t

        nc.sync.dma_start(w1_sb[:, 0:KB // 2, :], w1_r[:, 0:KB // 2, :])
        dma_x_sp(xq_pool, xq_sp, xq_r, "q", 0, halves=2)
        nc.sync.dma_start(w1_sb[:, KB // 2:KB, :], w1_r[:, KB // 2:KB, :])
        nc.sync.dma_start(b1_sb[:], b1[:])
        nc.sync.dma_start(w2_sb[:], w2_r[:])
        nc.sync.dma_start(b2_sb[:], b2[:])
        dma_x_sp(xk_pool, xk_sp, xk_r, "k", 0, halves=2)
        dma_x_sp(xk_pool, xk_sp, xk_r, "k", 1)
        dma_x_sp(xq_pool, xq_sp, xq_r, "q", 1)
        dma_x_sp(xk_pool, xk_sp, xk_r, "k", 2)
        dma_x_sp(xq_pool, xq_sp, xq_r, "q", 2)
        dma_x_sp(xk_pool, xk_sp, xk_r, "k", 3)
        nc.sync.dma_start(w3_sb[:], w3_r[:])
        nc.sync.dma_start(b3_sb[:], b3[:])
        for j in range(2):
            t = xv_pool.tile([P, KB // 2, S], BF, name=f"xv_{j}", tag="xv")
            nc.sync.dma_start(t[:], xv_r[:, j * (KB // 2):(j + 1) * (KB // 2), :])
            xv_t.append(t)
        dma_x_sp(xq_pool, xq_sp, xq_r, "q", 3)
        nc.sync.dma_start(w4_sb[:], w4_r[:])

        # constants (vector engine, cheap, up-front)
        warm = wpool.tile([P, QW], BF, name="warm", tag="warm")
        nc.vector.memset(warm[:], 0.0)
        nc.vector.memset(ones_row[:], 1.0)
        for s in range(SM):
            for h in range(H_CORE):
                nc.vector.memset(v_sb[s][:, h * (DK + 1) + DK: h * (DK + 1) + DK + 1], 1.0)

        # ---- emission-time progress state ----
        state = {
            "q_sp_done": 0,   # qT columns written for sp < this
            "k_sp_done": 0,
            "v_done": 0,      # v_sb[s] written for s < this
        }

        # ---- work generators (each yield = ~2 matmuls emitted) ----
        def qk_proj_gen(x_sp, w_sb, b_sb, dstT, sp, key):
            for m in range(MF):
                ps = psum.tile([P, QW], F32, name=f"pp_{key}_{m}_{sp}", tag="fill", bufs=2)
                for kb in range(KB):
                    nc.tensor.matmul(
                        ps[:],
                        w_sb[:, kb, m * P:(m + 1) * P],
                        x_sp[sp][:, kb, :],
                        start=(kb == 0),
                        stop=(kb == KB - 1),
                    )
                    if kb % 2 == 1:
                        yield 2
                nc.vector.tensor_scalar_add(
                    dstT[m][:, sp * QW:(sp + 1) * QW], ps[:], b_sb[:, m:m + 1]
                )
            state[key + "_sp_done"] = sp + 1
            yield 0

        def v_proj_gen(s2):
            # two s-tiles (s = 2*s2, 2*s2+1) share one [P, 512] psum
            ps = psum.tile([P, QW], F32, name=f"pv_{s2}", tag="fill", bufs=2)
            for i in range(2):
                s = 2 * s2 + i
                half = slice(i * F, (i + 1) * F)
                for kb in range(KB):
                    nc.tensor.matmul(
                        ps[0:P, half],
                        xv_t[kb // 4][:, kb % 4, s * P:(s + 1) * P],
                        w3_sb[:, kb, :],
                        start=(kb == 0),
                        stop=False,
                    )
                    if kb % 2 == 1:
                        yield 2
                nc.tensor.matmul(ps[0:P, half], ones_row[:], b3_sb[:],
                                 start=False, stop=True)
                yield 1
            for i in range(2):
                s = 2 * s2 + i
                for h in range(H_CORE):
                    nc.vector.tensor_copy(
                        v_sb[s][:, h * (DK + 1): h * (DK + 1) + DK],
                        ps[:, i * F + h * DK: i * F + (h + 1) * DK],
                    )
                state["v_done"] = s + 1
            yield 0

        def w4_gen(qt):
            for oc in range(D // QW):
                ps = psum.tile([P, QW], F32, name=f"po_{qt}_{oc}", tag="fill", bufs=2)
                for m in range(MF):
                    nc.tensor.matmul(
                        ps[:],
                        outT[m][:, qt * P:(qt + 1) * P],
                        w4_sb[:, m, oc * QW:(oc + 1) * QW],
                        start=(m == 0),
                        stop=(m == MF - 1),
                    )
                yield 2
                ob = stage.tile([P, QW], F32, name=f"ob_{qt}_{oc}", tag="ob", bufs=3)
                nc.vector.tensor_copy(ob[:], ps[:])
                nc.sync.dma_start(out[qt * P:(qt + 1) * P, oc * QW:(oc + 1) * QW], ob[:])
                yield 0

        # ---- attention emission ----
        # windows qw-major: (hp0,qw0), (hp1,qw0), (hp0,qw1), ...
        # scores for both heads of a pair land in one [P, 2*QW] psum tile
        # (2 banks) so ONE ACT exp covers both: halves the ACT instruction
        # count, saving the 172-cycle/instr PSUM access penalty.
        windows = [(hp, qw) for qw in range(NQW) for hp in range(MF)]
        NW = len(windows)
        sc_list = [(w, kt) for w in range(NW) for kt in range(SM)]
        NSCP = len(sc_list)                      # 128 score pairs
        pv_list = [(w, kt, h2) for w in range(NW) for kt in range(SM) for h2 in range(2)]
        NPV = len(pv_list)

        attn_tiles = {}
        pv_ps = {}
        normed = set()
        w4_ready = []   # qt indices whose w4 can run

        def emit_scores(w_idx, kt):
            hp, qw = windows[w_idx]
            ps = psum.tile([P, 2 * QW], F32, name=f"sc_{w_idx}_{kt}", tag="sc", bufs=2)
            for h2 in range(2):
                rsl = slice(h2 * DK, (h2 + 1) * DK)
                nc.tensor.matmul(
                    ps[:, h2 * QW:(h2 + 1) * QW],
                    kT[hp][rsl, kt * P:(kt + 1) * P],
                    qT[hp][rsl, qw * QW:(qw + 1) * QW],
                    start=True,
                    stop=True,
                )
            at = attn_pool.tile([P, 2 * QW], BF, name=f"at_{w_idx}_{kt}",
                                tag="attnT", bufs=ATTN_BUFS)
            nc.scalar.activation(
                at[:], ps[:], mybir.ActivationFunctionType.Exp,
                scale=1.0 / np.sqrt(DK),
            )
            attn_tiles[w_idx, kt] = at

        def emit_pv(w_idx, kt, h2):
            hp, qw = windows[w_idx]
            h = hp * 2 + h2
            key = (w_idx, h2)
            if key not in pv_ps:
                pv_ps[key] = psum.tile([P, QW], F32, name=f"pvps_{w_idx}_{h2}",
                                       tag="pv", bufs=2)
            vsl = slice(h * (DK + 1), h * (DK + 1) + DK + 1)
            at = attn_tiles[w_idx, kt] if h2 == 0 else attn_tiles.pop((w_idx, kt))
            nc.tensor.matmul(
                pv_ps[key][0:DK + 1, :],
                v_sb[kt][:, vsl],
                at[:, h2 * QW:(h2 + 1) * QW],
                start=(kt == 0),
                stop=(kt == SM - 1),
            )

        def emit_norm_h2(w_idx, h2):
            hp, qw = windows[w_idx]
            qsl = slice(qw * QW, (qw + 1) * QW)
            den = small.tile([1, QW], F32, name=f"den_{w_idx}_{h2}", tag="den", bufs=3)
            nc.vector.tensor_copy(den[:], pv_ps[w_idx, h2][DK:DK + 1, :])
            rec = small.tile([1, QW], F32, name=f"rec_{w_idx}_{h2}", tag="rec", bufs=3)
            nc.vector.reciprocal_approx_fast(rec[:], den[:])
            bc = small.tile([DK, QW], F32, name=f"bc_{w_idx}_{h2}", tag="bc", bufs=2)
            nc.gpsimd.partition_broadcast(bc[:], rec[:])
            raw = small.tile([DK, QW], BF, name=f"raw_{w_idx}_{h2}", tag="raw", bufs=4)
            nc.vector.tensor_copy(raw[:], pv_ps[w_idx, h2][0:DK, :])
            nc.vector.tensor_mul(
                outT[hp][h2 * DK:(h2 + 1) * DK, qsl], raw[:], bc[:]
            )
            if h2 == 1:
                normed.add(w_idx)
                # schedule w4 for this qw once both head-pair windows done
                base = 2 * qw
                if base in normed and base + 1 in normed:
                    for j in range(QW // P):
                        w4_ready.append(qw * (QW // P) + j)

        # ---- filler queue (priority order; v after xv/w3 have landed) ----
        filler_q = [
            qk_proj_gen(xk_sp, w2_sb, b2_sb, kT, 1, "k"),
            qk_proj_gen(xk_sp, w2_sb, b2_sb, kT, 2, "k"),
            qk_proj_gen(xk_sp, w2_sb, b2_sb, kT, 3, "k"),
            qk_proj_gen(xq_sp, w1_sb, b1_sb, qT, 1, "q"),
            v_proj_gen(0), v_proj_gen(1),
            qk_proj_gen(xq_sp, w1_sb, b1_sb, qT, 2, "q"),
            v_proj_gen(2), v_proj_gen(3), v_proj_gen(4),
            qk_proj_gen(xq_sp, w1_sb, b1_sb, qT, 3, "q"),
            v_proj_gen(5), v_proj_gen(6), v_proj_gen(7),
        ]
        filler_q = list(reversed(filler_q))  # pop() from the end
        w4_gens = []

        def drain_filler(budget):
            emitted = 0
            while emitted < budget:
                w4_ok = w4_ready and (len(w4_ready) > 10 or sc_i >= NSCP)
                if w4_gens:
                    gen = w4_gens[-1]
                elif w4_ok:
                    w4_gens.append(w4_gen(w4_ready.pop(0)))
                    gen = w4_gens[-1]
                elif filler_q:
                    gen = filler_q[-1]
                else:
                    return emitted
                try:
                    emitted += max(next(gen), 1)
                except StopIteration:
                    emitted += 1  # queue shrank: counts as scheduler progress
                    if w4_gens and gen is w4_gens[-1]:
                        w4_gens.pop()
                    elif filler_q and gen is filler_q[-1]:
                        filler_q.pop()
            return emitted

        # ---- HAM warmup: open the PE clock gate while input DMAs land ----
        for i in range(14):
            ps = psum.tile([P, QW], F32, name=f"wm_{i}", tag="fill", bufs=2)
            nc.tensor.matmul(ps[:], warm[:, 0:P], warm[:], start=True, stop=True)

        # ---- PRE: q/k projections for sp=0 (enables window 0) ----
        for _ in qk_proj_gen(xq_sp, w1_sb, b1_sb, qT, 0, "q"):
            pass
        for _ in qk_proj_gen(xk_sp, w2_sb, b2_sb, kT, 0, "k"):
            pass

        # ---- main scheduler loop ----
        sc_i = 0   # score PAIRS emitted
        pv_i = 0   # pv singles emitted
        while sc_i < NSCP or pv_i < NPV or filler_q or w4_gens or w4_ready:
            progress = 0
            # 1) PV drain (window-ordered, gated on v + margin behind scores)
            backlog = sc_i - pv_i // 2          # attn pair tiles in flight
            pv_budget = 3 if backlog > 6 else 2
            while pv_budget > 0 and pv_i < NPV:
                w_idx, kt, h2 = pv_list[pv_i]
                margin_ok = (sc_i - pv_i // 2) >= 3 or sc_i == NSCP
                if not margin_ok or state["v_done"] <= kt:
                    break
                emit_pv(w_idx, kt, h2)
                if kt == SM - 1:
                    emit_norm_h2(w_idx, h2)
                pv_i += 1
                pv_budget -= 1
                progress += 1
                if pv_i == NPV:
                    for i in range(12):
                        wps = psum.tile([P, QW], F32, name=f"wmt_{i}",
                                        tag="fill", bufs=2)
                        nc.tensor.matmul(wps[:], warm[:, 0:P], warm[:],
                                         start=True, stop=True)
            # 2) scores (gated on projection progress + attn backlog cap)
            if sc_i < NSCP and (sc_i - pv_i // 2) < ATTN_CAP:
                w_idx, kt = sc_list[sc_i]
                hp, qw = windows[w_idx]
                if state["k_sp_done"] > kt // (QW // P) and state["q_sp_done"] > qw:
                    emit_scores(w_idx, kt)
                    sc_i += 1
                    progress += 1
            # 3) filler
            progress += drain_filler(4)
            if progress == 0:
                # stuck: force whatever is forceable (pv without margin, then scores)
                if pv_i < NPV and pv_i // 2 < sc_i and state["v_done"] > pv_list[pv_i][1]:
                    w_idx, kt, h2 = pv_list[pv_i]
                    emit_pv(w_idx, kt, h2)
                    if kt == SM - 1:
                        emit_norm_h2(w_idx, h2)
                    pv_i += 1
                elif sc_i < NSCP:
                    w_idx, kt = sc_list[sc_i]
                    emit_scores(w_idx, kt)
                    sc_i += 1
                else:
                    raise RuntimeError("scheduler deadlock")


_NC_CACHE = None


def _get_nc():
    global _NC_CACHE
    if _NC_CACHE is None:
        _NC_CACHE = _build_kernel()
    return _NC_CACHE


def _make_in_maps(query, key, value, W1, b1, W2, b2, W3, b3, W4, b4):
    in_maps = []
    for c in range(N_CORES):
        b, g = divmod(c, 4)
        gs = slice(g * F, (g + 1) * F)
        in_maps.append({
            "xq_t": np.ascontiguousarray(query[b].T).astype(BF16),
            "xk_t": np.ascontiguousarray(key[b].T).astype(BF16),
            "xv_t": np.ascontiguousarray(value[b].T).astype(BF16),
            "w1t": np.ascontiguousarray(W1[gs, :].T).astype(BF16),
            "w2t": np.ascontiguousarray(W2[gs, :].T).astype(BF16),
            "w3t": np.ascontiguousarray(W3[gs, :].T).astype(BF16),
            "w4t": np.ascontiguousarray(W4[:, gs].T).astype(BF16),
            "b1c": np.ascontiguousarray(b1[gs].reshape(F // P, P).T).astype(np.float32),
            "b2c": np.ascontiguousarray(b2[gs].reshape(F // P, P).T).astype(np.float32),
            "b3r": b3[gs].reshape(1, F).astype(BF16),
        })
    return in_maps


def kernel(query, key, value, W1, b1, W2, b2, W3, b3, W4, b4, _trace=False, _tmpdir=None):
    args = [np.asarray(a) for a in (query, key, value, W1, b1, W2, b2, W3, b3, W4, b4)]
    nc = _get_nc()
    in_maps = _make_in_maps(*args)
    res = run_bass_kernel_spmd(
        nc, in_maps, core_ids=list(range(N_CORES)),
        trace=_trace, tmpdir=_tmpdir,
    )
    b4_f = args[10].astype(np.float32)
    full = np.zeros((B, S, D), np.float32)
    for c in range(N_CORES):
        full[c // 4] += res.results[c]["out"]
    full += b4_f[None, None, :]
    kernel.last_results = res
    return full


# revision 18
# speedup vs baseline: 1.0692x; 1.0692x over previous
"""Multi-head attention (B=2, S=2048, D=1024, H=16, d_k=64) on 8 NeuronCores.

Sharding: 8 cores = 2 batches x 4 head-groups (4 heads each).
Core c handles batch b = c//4 and heads 4*(c%4) .. 4*(c%4)+4 (feature
slice of width F=256). Each core computes its partial output-projection
contribution [S, D]; the host sums the 4 head-group partials per batch
and adds b4 (the "all-reduce" of the row-sharded W4 projection).

Device dataflow works in a "transposed world" so every matmul operand
is in its natural PE layout (contraction on partitions), with zero
on-device transposes:
  qT = W1g @ x_q.T  [F, S]   (lhsT = W1g.T host-prepped, rhs = x_q.T host-prepped)
  kT = W2g @ x_k.T  [F, S]
  v  = x_v @ W3g.T  [S, F]   (lhsT = x_v.T, rhs = W3g.T; bias via K=1 ones matmul)
  scoresT_h = kT_h.T @ qT_h        [S_keys, 512-q window]   (K = d_k = 64)
  attnT = exp(scoresT / 8)          ACT, PSUM->SBUF bf16, no max subtraction
  pv = [v_h | ones].T @ attnT      [65, 512]; row 64 = softmax denominator
  outT_h = pv[0:64] * (1/pv[64])   per-query normalization post-PV
  partial = outT.T @ W4g.T         [S, D]

All matmuls bf16 with f32 PSUM accumulation.

Schedule (HAM-aware): the TRN2 PE clock-gate (PE_HAM) halves the PE clock
whenever recent PE activity is low, and the attention inner loop alone
cannot keep it busy (scores+PV per key tile is ~0.9us of PE vs ~1.2us of
ACT exp). So ALL independent PE work - the q/k/v projections, and the W4
output projection of completed windows - is interleaved into the
scores/PV stream by an emission-time scheduler that keeps the in-order
PE queue dense: per tick it emits <=2 scores matmuls (gated on projection
progress + attn-tile backlog), <=3 PV matmuls (gated on v-projection
progress), and ~4 filler matmuls. DMA is panel-prioritized (w1/w2,
xq/xk first 512-col panels first) so the PE starts ~7us in, and xv/v
land in time for PV to chase scores with ~1 window of lag.
"""

import numpy as np
import ml_dtypes

import concourse.bass as bass
import concourse.mybir as mybir
import concourse.tile as tile
from concourse import bacc
from concourse.bass_utils import run_bass_kernel_spmd

BF16 = ml_dtypes.bfloat16
F32 = mybir.dt.float32
BF = mybir.dt.bfloat16

B, S, D = 2, 2048, 1024
H_CORE = 4          # heads per core
DK = 64             # head dim
F = H_CORE * DK     # features per core = 256
P = 128             # partitions
KB = D // P         # k blocks in D contraction = 8
SM = S // P         # seq tiles of 128 = 16
QW = 512            # query window width
NQW = S // QW       # query windows = 4
N_CORES = 8
ATTN_BUFS = 20      # attn sbuf PAIR tiles in flight (2KB/partition each)
ATTN_CAP = 17       # emission-time backlog cap (score pairs ahead of PV)


def _build_kernel():
    nc = bacc.Bacc(
        "TRN2",
        target_bir_lowering=False,
        debug=False,
        enable_asserts=False,
        num_devices=N_CORES,
    )

    xq = nc.dram_tensor("xq_t", [D, S], BF, kind="ExternalInput").ap()
    xk = nc.dram_tensor("xk_t", [D, S], BF, kind="ExternalInput").ap()
    xv = nc.dram_tensor("xv_t", [D, S], BF, kind="ExternalInput").ap()
    w1 = nc.dram_tensor("w1t", [D, F], BF, kind="ExternalInput").ap()
    w2 = nc.dram_tensor("w2t", [D, F], BF, kind="ExternalInput").ap()
    w3 = nc.dram_tensor("w3t", [D, F], BF, kind="ExternalInput").ap()
    w4 = nc.dram_tensor("w4t", [F, D], BF, kind="ExternalInput").ap()
    b1 = nc.dram_tensor("b1c", [P, F // P], F32, kind="ExternalInput").ap()
    b2 = nc.dram_tensor("b2c", [P, F // P], F32, kind="ExternalInput").ap()
    b3 = nc.dram_tensor("b3r", [1, F], BF, kind="ExternalInput").ap()
    out = nc.dram_tensor("out", [S, D], F32, kind="ExternalOutput").ap()

    with tile.TileContext(nc) as tc:
        _body(tc, xq, xk, xv, w1, w2, w3, w4, b1, b2, b3, out)

    nc.compile()
    return nc


def _body(tc, xq, xk, xv, w1, w2, w3, w4, b1, b2, b3, out):
    nc = tc.nc
    MF = F // P  # m tiles for the F=256 feature dim = 2

    with (
        tc.tile_pool(name="wpool", bufs=1) as wpool,
        tc.tile_pool(name="xqp", bufs=2) as xq_pool,
        tc.tile_pool(name="xkp", bufs=3) as xk_pool,
        tc.tile_pool(name="xvp", bufs=2) as xv_pool,
        tc.tile_pool(name="persist", bufs=1) as persist,
        tc.tile_pool(name="attn", bufs=ATTN_BUFS) as attn_pool,
        tc.tile_pool(name="small", bufs=4) as small,
        tc.tile_pool(name="stage", bufs=3) as stage,
        tc.tile_pool(name="psum", bufs=1, space="PSUM") as psum,
    ):
        # ---- weight / bias tiles (batched: one DMA per tensor) ----
        w1_sb = wpool.tile([P, KB, F], BF, name="w1_sb", tag="w1_sb")
        w2_sb = wpool.tile([P, KB, F], BF, name="w2_sb", tag="w2_sb")
        w3_sb = wpool.tile([P, KB, F], BF, name="w3_sb", tag="w3_sb")
        w4_sb = wpool.tile([P, MF, D], BF, name="w4_sb", tag="w4_sb")
        b1_sb = wpool.tile([P, MF], F32, name="b1_sb", tag="b1_sb")
        b2_sb = wpool.tile([P, MF], F32, name="b2_sb", tag="b2_sb")
        b3_sb = wpool.tile([1, F], BF, name="b3_sb", tag="b3_sb")
        ones_row = wpool.tile([1, P], BF, name="ones_row", tag="ones_row")

        # x: one [P, KB, QW] tile per query-window column panel; xv in 2 halves
        xq_sp = {}
        xk_sp = {}
        xv_t = []
        w1_r = w1.rearrange("(kb p) f -> p kb f", p=P)
        w2_r = w2.rearrange("(kb p) f -> p kb f", p=P)
        w3_r = w3.rearrange("(kb p) f -> p kb f", p=P)
        w4_r = w4.rearrange("(m p) d -> p m d", p=P)
        xq_r = xq.rearrange("(kb p) s -> p kb s", p=P)
        xk_r = xk.rearrange("(kb p) s -> p kb s", p=P)
        xv_r = xv.rearrange("(kb p) s -> p kb s", p=P)

        # persistent activations
        qT = [persist.tile([P, S], BF, name=f"qT_{m}", tag=f"qT_{m}") for m in range(MF)]
        kT = [persist.tile([P, S], BF, name=f"kT_{m}", tag=f"kT_{m}") for m in range(MF)]
        VW = H_CORE * (DK + 1)  # 260: per head h, cols 65h..65h+63 = v_h, col 65h+64 = 1
        v_sb = [persist.tile([P, VW], BF, name=f"v_{s}", tag=f"v_{s}") for s in range(SM)]
        outT = [persist.tile([P, S], BF, name=f"outT_{m}", tag=f"outT_{m}") for m in range(MF)]

        # ---- DMA emission, in need-order (few big transfers; the Sync
        # engine issues DMAs serially at ~0.6us each, so issue count matters)
        def dma_x_sp(pool, store, x_r, which, sp, halves=1):
            t = pool.tile([P, KB, QW], BF, name=f"x{which}_{sp}", tag=f"x{which}")
            hk = KB // halves
            for j in range(halves):
                nc.sync.dma_start(
                    t[:, j * hk:(j + 1) * hk, :],
                    x_r[:, j * hk:(j + 1) * hk, sp * QW:(sp + 1) * QW],
                )
            store[sp] = t

        xq0 = xq_pool.tile([P, KB, QW], BF, name="xq_0", tag="xq")
        xk0 = xk_pool.tile([P, KB, QW], BF, name="xk_0", tag="xk")
        xq_sp[0] = xq0
        xk_sp[0] = xk0
        HK = KB // 2
        nc.sync.dma_start(w1_sb[:, 0:HK, :], w1_r[:, 0:HK, :])
        nc.sync.dma_start(xq0[:, 0:HK, :], xq_r[:, 0:HK, 0:QW])
        nc.sync.dma_start(w1_sb[:, HK:KB, :], w1_r[:, HK:KB, :])
        nc.sync.dma_start(xq0[:, HK:KB, :], xq_r[:, HK:KB, 0:QW])
        nc.sync.dma_start(w2_sb[:, 0:HK, :], w2_r[:, 0:HK, :])
        nc.sync.dma_start(xk0[:, 0:HK, :], xk_r[:, 0:HK, 0:QW])
        nc.sync.dma_start(w2_sb[:, HK:KB, :], w2_r[:, HK:KB, :])
        nc.sync.dma_start(xk0[:, HK:KB, :], xk_r[:, HK:KB, 0:QW])
        nc.sync.dma_start(b1_sb[:], b1[:])
        nc.sync.dma_start(b2_sb[:], b2[:])
        dma_x_sp(xk_pool, xk_sp, xk_r, "k", 1)
        dma_x_sp(xq_pool, xq_sp, xq_r, "q", 1)
        dma_x_sp(xk_pool, xk_sp, xk_r, "k", 2)
        dma_x_sp(xq_pool, xq_sp, xq_r, "q", 2)
        dma_x_sp(xk_pool, xk_sp, xk_r, "k", 3)
        nc.sync.dma_start(w3_sb[:], w3_r[:])
        nc.sync.dma_start(b3_sb[:], b3[:])
        for j in range(2):
            t = xv_pool.tile([P, KB // 2, S], BF, name=f"xv_{j}", tag="xv")
            nc.sync.dma_start(t[:], xv_r[:, j * (KB // 2):(j + 1) * (KB // 2), :])
            xv_t.append(t)
        dma_x_sp(xq_pool, xq_sp, xq_r, "q", 3)
        nc.sync.dma_start(w4_sb[:], w4_r[:])

        # constants (vector engine, cheap, up-front)
        warm = wpool.tile([P, QW], BF, name="warm", tag="warm")
        nc.vector.memset(warm[:], 0.0)
        nc.vector.memset(ones_row[:], 1.0)
        for s in range(SM):
            for h in range(H_CORE):
                nc.vector.memset(v_sb[s][:, h * (DK + 1) + DK: h * (DK + 1) + DK + 1], 1.0)

        # ---- emission-time progress state ----
        state = {
            "q_sp_done": 0,   # qT columns written for sp < this
            "k_sp_done": 0,
            "v_done": 0,      # v_sb[s] written for s < this
        }

        # ---- work generators (each yield = ~2 matmuls emitted) ----
        def qk_proj_gen(x_sp, w_sb, b_sb, dstT, sp, key):
            for m in range(MF):
                ps = psum.tile([P, QW], F32, name=f"pp_{key}_{m}_{sp}", tag="fill", bufs=2)
                for kb in range(KB):
                    nc.tensor.matmul(
                        ps[:],
                        w_sb[:, kb, m * P:(m + 1) * P],
                        x_sp[sp][:, kb, :],
                        start=(kb == 0),
                        stop=(kb == KB - 1),
                    )
                    if kb % 2 == 1:
                        yield 2
                nc.vector.tensor_scalar_add(
                    dstT[m][:, sp * QW:(sp + 1) * QW], ps[:], b_sb[:, m:m + 1]
                )
            state[key + "_sp_done"] = sp + 1
            yield 0

        def v_proj_gen(s2):
            # two s-tiles (s = 2*s2, 2*s2+1) share one [P, 512] psum
            ps = psum.tile([P, QW], F32, name=f"pv_{s2}", tag="fill", bufs=2)
            for i in range(2):
                s = 2 * s2 + i
                half = slice(i * F, (i + 1) * F)
                for kb in range(KB):
                    nc.tensor.matmul(
                        ps[0:P, half],
                        xv_t[kb // 4][:, kb % 4, s * P:(s + 1) * P],
                        w3_sb[:, kb, :],
                        start=(kb == 0),
                        stop=False,
                    )
                    if kb % 2 == 1:
                        yield 2
                nc.tensor.matmul(ps[0:P, half], ones_row[:], b3_sb[:],
                                 start=False, stop=True)
                yield 1
            for i in range(2):
                s = 2 * s2 + i
                for h in range(H_CORE):
                    nc.vector.tensor_copy(
                        v_sb[s][:, h * (DK + 1): h * (DK + 1) + DK],
                        ps[:, i * F + h * DK: i * F + (h + 1) * DK],
                    )
                state["v_done"] = s + 1
            yield 0

        def w4_gen(qt):
            for oc in range(D // QW):
                ps = psum.tile([P, QW], F32, name=f"po_{qt}_{oc}", tag="fill", bufs=2)
                for m in range(MF):
                    nc.tensor.matmul(
                        ps[:],
                        outT[m][:, qt * P:(qt + 1) * P],
                        w4_sb[:, m, oc * QW:(oc + 1) * QW],
                        start=(m == 0),
                        stop=(m == MF - 1),
                    )
                yield 2
                ob = stage.tile([P, QW], F32, name=f"ob_{qt}_{oc}", tag="ob", bufs=3)
                nc.vector.tensor_copy(ob[:], ps[:])
                nc.sync.dma_start(out[qt * P:(qt + 1) * P, oc * QW:(oc + 1) * QW], ob[:])
                yield 0

        # ---- attention emission ----
        # windows qw-major: (hp0,qw0), (hp1,qw0), (hp0,qw1), ...
        # scores for both heads of a pair land in one [P, 2*QW] psum tile
        # (2 banks) so ONE ACT exp covers both: halves the ACT instruction
        # count, saving the 172-cycle/instr PSUM access penalty.
        windows = [(hp, qw) for qw in range(NQW) for hp in range(MF)]
        NW = len(windows)
        sc_list = [(w, kt) for w in range(NW) for kt in range(SM)]
        NSCP = len(sc_list)                      # 128 score pairs
        pv_list = [(w, kt, h2) for w in range(NW) for kt in range(SM) for h2 in range(2)]
        NPV = len(pv_list)

        attn_tiles = {}
        pv_ps = {}
        normed = set()
        w4_ready = []   # qt indices whose w4 can run

        def emit_scores(w_idx, kt):
            hp, qw = windows[w_idx]
            ps = psum.tile([P, 2 * QW], F32, name=f"sc_{w_idx}_{kt}", tag="sc", bufs=2)
            for h2 in range(2):
                rsl = slice(h2 * DK, (h2 + 1) * DK)
                nc.tensor.matmul(
                    ps[:, h2 * QW:(h2 + 1) * QW],
                    kT[hp][rsl, kt * P:(kt + 1) * P],
                    qT[hp][rsl, qw * QW:(qw + 1) * QW],
                    start=True,
                    stop=True,
                )
            at = attn_pool.tile([P, 2 * QW], BF, name=f"at_{w_idx}_{kt}",
                                tag="attnT", bufs=ATTN_BUFS)
            nc.scalar.activation(
                at[:], ps[:], mybir.ActivationFunctionType.Exp,
                scale=1.0 / np.sqrt(DK),
            )
            attn_tiles[w_idx, kt] = at

        def emit_pv(w_idx, kt, h2):
            hp, qw = windows[w_idx]
            h = hp * 2 + h2
            key = (w_idx, h2)
            if key not in pv_ps:
                pv_ps[key] = psum.tile([P, QW], F32, name=f"pvps_{w_idx}_{h2}",
                                       tag="pv", bufs=2)
            vsl = slice(h * (DK + 1), h * (DK + 1) + DK + 1)
            at = attn_tiles[w_idx, kt] if h2 == 0 else attn_tiles.pop((w_idx, kt))
            nc.tensor.matmul(
                pv_ps[key][0:DK + 1, :],
                v_sb[kt][:, vsl],
                at[:, h2 * QW:(h2 + 1) * QW],
                start=(kt == 0),
                stop=(kt == SM - 1),
            )

        def emit_norm_h2(w_idx, h2):
            hp, qw = windows[w_idx]
            qsl = slice(qw * QW, (qw + 1) * QW)
            den = small.tile([1, QW], F32, name=f"den_{w_idx}_{h2}", tag="den", bufs=3)
            nc.vector.tensor_copy(den[:], pv_ps[w_idx, h2][DK:DK + 1, :])
            rec = small.tile([1, QW], F32, name=f"rec_{w_idx}_{h2}", tag="rec", bufs=3)
            nc.vector.reciprocal_approx_fast(rec[:], den[:])
            bc = small.tile([DK, QW], F32, name=f"bc_{w_idx}_{h2}", tag="bc", bufs=2)
            nc.gpsimd.partition_broadcast(bc[:], rec[:])
            raw = small.tile([DK, QW], BF, name=f"raw_{w_idx}_{h2}", tag="raw", bufs=4)
            nc.vector.tensor_copy(raw[:], pv_ps[w_idx, h2][0:DK, :])
            nc.vector.tensor_mul(
                outT[hp][h2 * DK:(h2 + 1) * DK, qsl], raw[:], bc[:]
            )
            if h2 == 1:
                normed.add(w_idx)
                # schedule w4 for this qw once both head-pair windows done
                base = 2 * qw
                if base in normed and base + 1 in normed:
                    for j in range(QW // P):
                        w4_ready.append(qw * (QW // P) + j)

        # ---- filler queue (priority order; v after xv/w3 have landed) ----
        filler_q = [
            qk_proj_gen(xk_sp, w2_sb, b2_sb, kT, 1, "k"),
            qk_proj_gen(xk_sp, w2_sb, b2_sb, kT, 2, "k"),
            qk_proj_gen(xk_sp, w2_sb, b2_sb, kT, 3, "k"),
            qk_proj_gen(xq_sp, w1_sb, b1_sb, qT, 1, "q"),
            v_proj_gen(0), v_proj_gen(1),
            qk_proj_gen(xq_sp, w1_sb, b1_sb, qT, 2, "q"),
            v_proj_gen(2), v_proj_gen(3), v_proj_gen(4),
            qk_proj_gen(xq_sp, w1_sb, b1_sb, qT, 3, "q"),
            v_proj_gen(5), v_proj_gen(6), v_proj_gen(7),
        ]
        filler_q = list(reversed(filler_q))  # pop() from the end
        w4_gens = []

        def drain_filler(budget):
            emitted = 0
            while emitted < budget:
                w4_ok = w4_ready and (len(w4_ready) > 10 or sc_i >= NSCP)
                if w4_gens:
                    gen = w4_gens[-1]
                elif w4_ok:
                    w4_gens.append(w4_gen(w4_ready.pop(0)))
                    gen = w4_gens[-1]
                elif filler_q:
                    gen = filler_q[-1]
                else:
                    return emitted
                try:
                    emitted += max(next(gen), 1)
                except StopIteration:
                    emitted += 1  # queue shrank: counts as scheduler progress
                    if w4_gens and gen is w4_gens[-1]:
                        w4_gens.pop()
                    elif filler_q and gen is filler_q[-1]:
                        filler_q.pop()
            return emitted

        # ---- HAM warmup: open the PE clock gate while input DMAs land ----
        for i in range(14):
            ps = psum.tile([P, QW], F32, name=f"wm_{i}", tag="fill", bufs=2)
            nc.tensor.matmul(ps[:], warm[:, 0:P], warm[:], start=True, stop=True)

        # ---- PRE: q/k projections for sp=0 (enables window 0) ----
        for _ in qk_proj_gen(xq_sp, w1_sb, b1_sb, qT, 0, "q"):
            pass
        for _ in qk_proj_gen(xk_sp, w2_sb, b2_sb, kT, 0, "k"):
            pass

        # ---- main scheduler loop ----
        sc_i = 0   # score PAIRS emitted
        pv_i = 0   # pv singles emitted
        while sc_i < NSCP or pv_i < NPV or filler_q or w4_gens or w4_ready:
            progress = 0
            # 1) PV drain (window-ordered, gated on v + margin behind scores)
            backlog = sc_i - pv_i // 2          # attn pair tiles in flight
            pv_budget = 3 if backlog > 6 else 2
            while pv_budget > 0 and pv_i < NPV:
                w_idx, kt, h2 = pv_list[pv_i]
                margin_ok = (sc_i - pv_i // 2) >= 3 or sc_i == NSCP
                if not margin_ok or state["v_done"] <= kt:
                    break
                emit_pv(w_idx, kt, h2)
                if kt == SM - 1:
                    emit_norm_h2(w_idx, h2)
                pv_i += 1
                pv_budget -= 1
                progress += 1
            # 2) scores (gated on projection progress + attn backlog cap)
            if sc_i < NSCP and (sc_i - pv_i // 2) < ATTN_CAP:
                w_idx, kt = sc_list[sc_i]
                hp, qw = windows[w_idx]
                if state["k_sp_done"] > kt // (QW // P) and state["q_sp_done"] > qw:
                    emit_scores(w_idx, kt)
                    sc_i += 1
                    progress += 1
            # 3) filler
            progress += drain_filler(2 if sc_i >= NSCP else 4)
            if progress == 0:
                # stuck: force whatever is forceable (pv without margin, then scores)
                if pv_i < NPV and pv_i // 2 < sc_i and state["v_done"] > pv_list[pv_i][1]:
                    w_idx, kt, h2 = pv_list[pv_i]
                    emit_pv(w_idx, kt, h2)
                    if kt == SM - 1:
                        emit_norm_h2(w_idx, h2)
                    pv_i += 1
                elif sc_i < NSCP:
                    w_idx, kt = sc_list[sc_i]
                    emit_scores(w_idx, kt)
                    sc_i += 1
                else:
                    raise RuntimeError("scheduler deadlock")


_NC_CACHE = None


def _get_nc():
    global _NC_CACHE
    if _NC_CACHE is None:
        _NC_CACHE = _build_kernel()
    return _NC_CACHE


def _make_in_maps(query, key, value, W1, b1, W2, b2, W3, b3, W4, b4):
    in_maps = []
    for c in range(N_CORES):
        b, g = divmod(c, 4)
        gs = slice(g * F, (g + 1) * F)
        in_maps.append({
            "xq_t": np.ascontiguousarray(query[b].T).astype(BF16),
            "xk_t": np.ascontiguousarray(key[b].T).astype(BF16),
            "xv_t": np.ascontiguousarray(value[b].T).astype(BF16),
            "w1t": np.ascontiguousarray(W1[gs, :].T).astype(BF16),
            "w2t": np.ascontiguousarray(W2[gs, :].T).astype(BF16),
            "w3t": np.ascontiguousarray(W3[gs, :].T).astype(BF16),
            "w4t": np.ascontiguousarray(W4[:, gs].T).astype(BF16),
            "b1c": np.ascontiguousarray(b1[gs].reshape(F // P, P).T).astype(np.float32),
            "b2c": np.ascontiguousarray(b2[gs].reshape(F // P, P).T).astype(np.float32),
            "b3r": b3[gs].reshape(1, F).astype(BF16),
        })
    return in_maps


def kernel(query, key, value, W1, b1, W2, b2, W3, b3, W4, b4, _trace=False, _tmpdir=None):
    args = [np.asarray(a) for a in (query, key, value, W1, b1, W2, b2, W3, b3, W4, b4)]
    nc = _get_nc()
    in_maps = _make_in_maps(*args)
    res = run_bass_kernel_spmd(
        nc, in_maps, core_ids=list(range(N_CORES)),
        trace=_trace, tmpdir=_tmpdir,
    )
    b4_f = args[10].astype(np.float32)
    full = np.zeros((B, S, D), np.float32)
    for c in range(N_CORES):
        full[c // 4] += res.results[c]["out"]
    full += b4_f[None, None, :]
    kernel.last_results = res
    return full


# revision 20
# speedup vs baseline: 1.0733x; 1.0038x over previous
"""Multi-head attention (B=2, S=2048, D=1024, H=16, d_k=64) on 8 NeuronCores.

Sharding: 8 cores = 2 batches x 4 head-groups (4 heads each).
Core c handles batch b = c//4 and heads 4*(c%4) .. 4*(c%4)+4 (feature
slice of width F=256). Each core computes its partial output-projection
contribution [S, D]; the host sums the 4 head-group partials per batch
and adds b4 (the "all-reduce" of the row-sharded W4 projection).

Device dataflow works in a "transposed world" so every matmul operand
is in its natural PE layout (contraction on partitions), with zero
on-device transposes:
  qT = W1g @ x_q.T  [F, S]   (lhsT = W1g.T host-prepped, rhs = x_q.T host-prepped)
  kT = W2g @ x_k.T  [F, S]
  v  = x_v @ W3g.T  [S, F]   (lhsT = x_v.T, rhs = W3g.T; bias via K=1 ones matmul)
  scoresT_h = kT_h.T @ qT_h        [S_keys, 512-q window]   (K = d_k = 64)
  attnT = exp(scoresT / 8)          ACT, PSUM->SBUF bf16, no max subtraction
  pv = [v_h | ones].T @ attnT      [65, 512]; row 64 = softmax denominator
  outT_h = pv[0:64] * (1/pv[64])   per-query normalization post-PV
  partial = outT.T @ W4g.T         [S, D]

All matmuls bf16 with f32 PSUM accumulation.

Schedule (HAM-aware): the TRN2 PE clock-gate (PE_HAM) halves the PE clock
whenever recent PE activity is low, and the attention inner loop alone
cannot keep it busy (scores+PV per key tile is ~0.9us of PE vs ~1.2us of
ACT exp). So ALL independent PE work - the q/k/v projections, and the W4
output projection of completed windows - is interleaved into the
scores/PV stream by an emission-time scheduler that keeps the in-order
PE queue dense: per tick it emits <=2 scores matmuls (gated on projection
progress + attn-tile backlog), <=3 PV matmuls (gated on v-projection
progress), and ~4 filler matmuls. DMA is panel-prioritized (w1/w2,
xq/xk first 512-col panels first) so the PE starts ~7us in, and xv/v
land in time for PV to chase scores with ~1 window of lag.
"""

import numpy as np
import ml_dtypes

import concourse.bass as bass
import concourse.mybir as mybir
import concourse.tile as tile
from concourse import bacc
from concourse.bass_utils import run_bass_kernel_spmd

BF16 = ml_dtypes.bfloat16
F32 = mybir.dt.float32
BF = mybir.dt.bfloat16

B, S, D = 2, 2048, 1024
H_CORE = 4          # heads per core
DK = 64             # head dim
F = H_CORE * DK     # features per core = 256
P = 128             # partitions
KB = D // P         # k blocks in D contraction = 8
SM = S // P         # seq tiles of 128 = 16
QW = 512            # query window width
NQW = S // QW       # query windows = 4
N_CORES = 8
ATTN_BUFS = 20      # attn sbuf PAIR tiles in flight (2KB/partition each)
ATTN_CAP = 17       # emission-time backlog cap (score pairs ahead of PV)


def _build_kernel():
    nc = bacc.Bacc(
        "TRN2",
        target_bir_lowering=False,
        debug=False,
        enable_asserts=False,
        num_devices=N_CORES,
    )

    xq = nc.dram_tensor("xq_t", [D, S], BF, kind="ExternalInput").ap()
    xk = nc.dram_tensor("xk_t", [D, S], BF, kind="ExternalInput").ap()
    xv = nc.dram_tensor("xv_t", [D, S], BF, kind="ExternalInput").ap()
    w1 = nc.dram_tensor("w1t", [D, F], BF, kind="ExternalInput").ap()
    w2 = nc.dram_tensor("w2t", [D, F], BF, kind="ExternalInput").ap()
    w3 = nc.dram_tensor("w3t", [D, F], BF, kind="ExternalInput").ap()
    w4 = nc.dram_tensor("w4t", [F, D], BF, kind="ExternalInput").ap()
    b1 = nc.dram_tensor("b1c", [P, F // P], F32, kind="ExternalInput").ap()
    b2 = nc.dram_tensor("b2c", [P, F // P], F32, kind="ExternalInput").ap()
    b3 = nc.dram_tensor("b3r", [1, F], BF, kind="ExternalInput").ap()
    out = nc.dram_tensor("out", [S, D], BF, kind="ExternalOutput").ap()

    with tile.TileContext(nc) as tc:
        _body(tc, xq, xk, xv, w1, w2, w3, w4, b1, b2, b3, out)

    nc.compile()
    return nc


def _body(tc, xq, xk, xv, w1, w2, w3, w4, b1, b2, b3, out):
    nc = tc.nc
    MF = F // P  # m tiles for the F=256 feature dim = 2

    with (
        tc.tile_pool(name="wpool", bufs=1) as wpool,
        tc.tile_pool(name="xqp", bufs=2) as xq_pool,
        tc.tile_pool(name="xkp", bufs=3) as xk_pool,
        tc.tile_pool(name="xvp", bufs=2) as xv_pool,
        tc.tile_pool(name="persist", bufs=1) as persist,
        tc.tile_pool(name="attn", bufs=ATTN_BUFS) as attn_pool,
        tc.tile_pool(name="small", bufs=4) as small,
        tc.tile_pool(name="stage", bufs=3) as stage,
        tc.tile_pool(name="psum", bufs=1, space="PSUM") as psum,
    ):
        # ---- weight / bias tiles (batched: one DMA per tensor) ----
        w1_sb = wpool.tile([P, KB, F], BF, name="w1_sb", tag="w1_sb")
        w2_sb = wpool.tile([P, KB, F], BF, name="w2_sb", tag="w2_sb")
        w3_sb = wpool.tile([P, KB, F], BF, name="w3_sb", tag="w3_sb")
        w4_sb = wpool.tile([P, MF, D], BF, name="w4_sb", tag="w4_sb")
        b1_sb = wpool.tile([P, MF], F32, name="b1_sb", tag="b1_sb")
        b2_sb = wpool.tile([P, MF], F32, name="b2_sb", tag="b2_sb")
        b3_sb = wpool.tile([1, F], BF, name="b3_sb", tag="b3_sb")
        ones_row = wpool.tile([1, P], BF, name="ones_row", tag="ones_row")

        # x: one [P, KB, QW] tile per query-window column panel; xv in 2 halves
        xq_sp = {}
        xk_sp = {}
        xv_t = []
        w1_r = w1.rearrange("(kb p) f -> p kb f", p=P)
        w2_r = w2.rearrange("(kb p) f -> p kb f", p=P)
        w3_r = w3.rearrange("(kb p) f -> p kb f", p=P)
        w4_r = w4.rearrange("(m p) d -> p m d", p=P)
        xq_r = xq.rearrange("(kb p) s -> p kb s", p=P)
        xk_r = xk.rearrange("(kb p) s -> p kb s", p=P)
        xv_r = xv.rearrange("(kb p) s -> p kb s", p=P)

        # persistent activations
        qT = [persist.tile([P, S], BF, name=f"qT_{m}", tag=f"qT_{m}") for m in range(MF)]
        kT = [persist.tile([P, S], BF, name=f"kT_{m}", tag=f"kT_{m}") for m in range(MF)]
        VW = H_CORE * (DK + 1)  # 260: per head h, cols 65h..65h+63 = v_h, col 65h+64 = 1
        v_sb = [persist.tile([P, VW], BF, name=f"v_{s}", tag=f"v_{s}") for s in range(SM)]
        outT = [persist.tile([P, S], BF, name=f"outT_{m}", tag=f"outT_{m}") for m in range(MF)]

        # ---- DMA emission, in need-order (few big transfers; the Sync
        # engine issues DMAs serially at ~0.6us each, so issue count matters)
        def dma_x_sp(pool, store, x_r, which, sp, halves=1):
            t = pool.tile([P, KB, QW], BF, name=f"x{which}_{sp}", tag=f"x{which}")
            hk = KB // halves
            for j in range(halves):
                nc.sync.dma_start(
                    t[:, j * hk:(j + 1) * hk, :],
                    x_r[:, j * hk:(j + 1) * hk, sp * QW:(sp + 1) * QW],
                )
            store[sp] = t

        nc.sync.dma_start(w1_sb[:, 0:KB // 2, :], w1_r[:, 0:KB // 2, :])
        dma_x_sp(xq_pool, xq_sp, xq_r, "q", 0, halves=2)
        nc.sync.dma_start(w1_sb[:, KB // 2:KB, :], w1_r[:, KB // 2:KB, :])
        nc.sync.dma_start(b1_sb[:], b1[:])
        nc.sync.dma_start(w2_sb[:], w2_r[:])
        nc.sync.dma_start(b2_sb[:], b2[:])
        dma_x_sp(xk_pool, xk_sp, xk_r, "k", 0, halves=2)
        dma_x_sp(xk_pool, xk_sp, xk_r, "k", 1)
        dma_x_sp(xq_pool, xq_sp, xq_r, "q", 1)
        dma_x_sp(xk_pool, xk_sp, xk_r, "k", 2)
        dma_x_sp(xq_pool, xq_sp, xq_r, "q", 2)
        dma_x_sp(xk_pool, xk_sp, xk_r, "k", 3)
        nc.sync.dma_start(w3_sb[:], w3_r[:])
        nc.sync.dma_start(b3_sb[:], b3[:])
        for j in range(2):
            t = xv_pool.tile([P, KB // 2, S], BF, name=f"xv_{j}", tag="xv")
            nc.sync.dma_start(t[:], xv_r[:, j * (KB // 2):(j + 1) * (KB // 2), :])
            xv_t.append(t)
        dma_x_sp(xq_pool, xq_sp, xq_r, "q", 3)
        nc.sync.dma_start(w4_sb[:], w4_r[:])

        # constants (vector engine, cheap, up-front)
        warm = wpool.tile([P, QW], BF, name="warm", tag="warm")
        nc.vector.memset(warm[:], 0.0)
        nc.vector.memset(ones_row[:], 1.0)
        for s in range(SM):
            for h in range(H_CORE):
                nc.vector.memset(v_sb[s][:, h * (DK + 1) + DK: h * (DK + 1) + DK + 1], 1.0)

        # ---- emission-time progress state ----
        state = {
            "q_sp_done": 0,   # qT columns written for sp < this
            "k_sp_done": 0,
            "v_done": 0,      # v_sb[s] written for s < this
        }

        # ---- work generators (each yield = ~2 matmuls emitted) ----
        def qk_proj_gen(x_sp, w_sb, b_sb, dstT, sp, key):
            for m in range(MF):
                ps = psum.tile([P, QW], F32, name=f"pp_{key}_{m}_{sp}", tag="fill", bufs=2)
                for kb in range(KB):
                    nc.tensor.matmul(
                        ps[:],
                        w_sb[:, kb, m * P:(m + 1) * P],
                        x_sp[sp][:, kb, :],
                        start=(kb == 0),
                        stop=(kb == KB - 1),
                    )
                    if kb % 2 == 1:
                        yield 2
                nc.vector.tensor_scalar_add(
                    dstT[m][:, sp * QW:(sp + 1) * QW], ps[:], b_sb[:, m:m + 1]
                )
            state[key + "_sp_done"] = sp + 1
            yield 0

        def v_proj_gen(s2):
            # two s-tiles (s = 2*s2, 2*s2+1) share one [P, 512] psum
            ps = psum.tile([P, QW], F32, name=f"pv_{s2}", tag="fill", bufs=2)
            for i in range(2):
                s = 2 * s2 + i
                half = slice(i * F, (i + 1) * F)
                for kb in range(KB):
                    nc.tensor.matmul(
                        ps[0:P, half],
                        xv_t[kb // 4][:, kb % 4, s * P:(s + 1) * P],
                        w3_sb[:, kb, :],
                        start=(kb == 0),
                        stop=False,
                    )
                    if kb % 2 == 1:
                        yield 2
                nc.tensor.matmul(ps[0:P, half], ones_row[:], b3_sb[:],
                                 start=False, stop=True)
                yield 1
            for i in range(2):
                s = 2 * s2 + i
                for h in range(H_CORE):
                    nc.vector.tensor_copy(
                        v_sb[s][:, h * (DK + 1): h * (DK + 1) + DK],
                        ps[:, i * F + h * DK: i * F + (h + 1) * DK],
                    )
                state["v_done"] = s + 1
            yield 0

        def w4_gen(qt):
            for oc in range(D // QW):
                ps = psum.tile([P, QW], F32, name=f"po_{qt}_{oc}", tag="fill", bufs=2)
                for m in range(MF):
                    nc.tensor.matmul(
                        ps[:],
                        outT[m][:, qt * P:(qt + 1) * P],
                        w4_sb[:, m, oc * QW:(oc + 1) * QW],
                        start=(m == 0),
                        stop=(m == MF - 1),
                    )
                yield 2
                ob = stage.tile([P, QW], BF, name=f"ob_{qt}_{oc}", tag="ob", bufs=3)
                nc.vector.tensor_copy(ob[:], ps[:])
                nc.sync.dma_start(out[qt * P:(qt + 1) * P, oc * QW:(oc + 1) * QW], ob[:])
                yield 0

        # ---- attention emission ----
        # windows qw-major: (hp0,qw0), (hp1,qw0), (hp0,qw1), ...
        # scores for both heads of a pair land in one [P, 2*QW] psum tile
        # (2 banks) so ONE ACT exp covers both: halves the ACT instruction
        # count, saving the 172-cycle/instr PSUM access penalty.
        windows = [(hp, qw) for qw in range(NQW) for hp in range(MF)]
        NW = len(windows)
        sc_list = [(w, kt) for w in range(NW) for kt in range(SM)]
        NSCP = len(sc_list)                      # 128 score pairs
        pv_list = [(w, kt, h2) for w in range(NW) for kt in range(SM) for h2 in range(2)]
        NPV = len(pv_list)

        attn_tiles = {}
        pv_ps = {}
        normed = set()
        w4_ready = []   # qt indices whose w4 can run

        def emit_scores(w_idx, kt):
            hp, qw = windows[w_idx]
            ps = psum.tile([P, 2 * QW], F32, name=f"sc_{w_idx}_{kt}", tag="sc", bufs=2)
            for h2 in range(2):
                rsl = slice(h2 * DK, (h2 + 1) * DK)
                nc.tensor.matmul(
                    ps[:, h2 * QW:(h2 + 1) * QW],
                    kT[hp][rsl, kt * P:(kt + 1) * P],
                    qT[hp][rsl, qw * QW:(qw + 1) * QW],
                    start=True,
                    stop=True,
                )
            at = attn_pool.tile([P, 2 * QW], BF, name=f"at_{w_idx}_{kt}",
                                tag="attnT", bufs=ATTN_BUFS)
            nc.scalar.activation(
                at[:], ps[:], mybir.ActivationFunctionType.Exp,
                scale=1.0 / np.sqrt(DK),
            )
            attn_tiles[w_idx, kt] = at

        def emit_pv(w_idx, kt, h2):
            hp, qw = windows[w_idx]
            h = hp * 2 + h2
            key = (w_idx, h2)
            if key not in pv_ps:
                pv_ps[key] = psum.tile([P, QW], F32, name=f"pvps_{w_idx}_{h2}",
                                       tag="pv", bufs=2)
            vsl = slice(h * (DK + 1), h * (DK + 1) + DK + 1)
            at = attn_tiles[w_idx, kt] if h2 == 0 else attn_tiles.pop((w_idx, kt))
            nc.tensor.matmul(
                pv_ps[key][0:DK + 1, :],
                v_sb[kt][:, vsl],
                at[:, h2 * QW:(h2 + 1) * QW],
                start=(kt == 0),
                stop=(kt == SM - 1),
            )

        def emit_norm_h2(w_idx, h2):
            hp, qw = windows[w_idx]
            qsl = slice(qw * QW, (qw + 1) * QW)
            den = small.tile([1, QW], F32, name=f"den_{w_idx}_{h2}", tag="den", bufs=3)
            nc.vector.tensor_copy(den[:], pv_ps[w_idx, h2][DK:DK + 1, :])
            rec = small.tile([1, QW], F32, name=f"rec_{w_idx}_{h2}", tag="rec", bufs=3)
            nc.vector.reciprocal_approx_fast(rec[:], den[:])
            bc = small.tile([DK, QW], F32, name=f"bc_{w_idx}_{h2}", tag="bc", bufs=2)
            nc.gpsimd.partition_broadcast(bc[:], rec[:])
            raw = small.tile([DK, QW], BF, name=f"raw_{w_idx}_{h2}", tag="raw", bufs=4)
            nc.vector.tensor_copy(raw[:], pv_ps[w_idx, h2][0:DK, :])
            nc.vector.tensor_mul(
                outT[hp][h2 * DK:(h2 + 1) * DK, qsl], raw[:], bc[:]
            )
            if h2 == 1:
                normed.add(w_idx)
                # schedule w4 for this qw once both head-pair windows done
                base = 2 * qw
                if base in normed and base + 1 in normed:
                    for j in range(QW // P):
                        w4_ready.append(qw * (QW // P) + j)

        # ---- filler queue (priority order; v after xv/w3 have landed) ----
        filler_q = [
            qk_proj_gen(xk_sp, w2_sb, b2_sb, kT, 1, "k"),
            qk_proj_gen(xk_sp, w2_sb, b2_sb, kT, 2, "k"),
            qk_proj_gen(xk_sp, w2_sb, b2_sb, kT, 3, "k"),
            qk_proj_gen(xq_sp, w1_sb, b1_sb, qT, 1, "q"),
            v_proj_gen(0), v_proj_gen(1),
            qk_proj_gen(xq_sp, w1_sb, b1_sb, qT, 2, "q"),
            v_proj_gen(2), v_proj_gen(3), v_proj_gen(4),
            qk_proj_gen(xq_sp, w1_sb, b1_sb, qT, 3, "q"),
            v_proj_gen(5), v_proj_gen(6), v_proj_gen(7),
        ]
        filler_q = list(reversed(filler_q))  # pop() from the end
        w4_gens = []

        def drain_filler(budget):
            emitted = 0
            while emitted < budget:
                w4_ok = w4_ready and (len(w4_ready) > 10 or sc_i >= NSCP)
                if w4_gens:
                    gen = w4_gens[-1]
                elif w4_ok:
                    w4_gens.append(w4_gen(w4_ready.pop(0)))
                    gen = w4_gens[-1]
                elif filler_q:
                    gen = filler_q[-1]
                else:
                    return emitted
                try:
                    emitted += max(next(gen), 1)
                except StopIteration:
                    emitted += 1  # queue shrank: counts as scheduler progress
                    if w4_gens and gen is w4_gens[-1]:
                        w4_gens.pop()
                    elif filler_q and gen is filler_q[-1]:
                        filler_q.pop()
            return emitted

        # ---- HAM warmup: open the PE clock gate while input DMAs land ----
        for i in range(14):
            ps = psum.tile([P, QW], F32, name=f"wm_{i}", tag="fill", bufs=2)
            nc.tensor.matmul(ps[:], warm[:, 0:P], warm[:], start=True, stop=True)

        # ---- PRE: q/k projections for sp=0 (enables window 0) ----
        for _ in qk_proj_gen(xq_sp, w1_sb, b1_sb, qT, 0, "q"):
            pass
        for _ in qk_proj_gen(xk_sp, w2_sb, b2_sb, kT, 0, "k"):
            pass

        # ---- main scheduler loop ----
        sc_i = 0   # score PAIRS emitted
        pv_i = 0   # pv singles emitted
        while sc_i < NSCP or pv_i < NPV or filler_q or w4_gens or w4_ready:
            progress = 0
            # 1) PV drain (window-ordered, gated on v + margin behind scores)
            backlog = sc_i - pv_i // 2          # attn pair tiles in flight
            pv_budget = 3 if backlog > 6 else 2
            while pv_budget > 0 and pv_i < NPV:
                w_idx, kt, h2 = pv_list[pv_i]
                margin_ok = (sc_i - pv_i // 2) >= 3 or sc_i == NSCP
                if not margin_ok or state["v_done"] <= kt:
                    break
                emit_pv(w_idx, kt, h2)
                if kt == SM - 1:
                    emit_norm_h2(w_idx, h2)
                pv_i += 1
                pv_budget -= 1
                progress += 1
            # 2) scores (gated on projection progress + attn backlog cap)
            if sc_i < NSCP and (sc_i - pv_i // 2) < ATTN_CAP:
                w_idx, kt = sc_list[sc_i]
                hp, qw = windows[w_idx]
                if state["k_sp_done"] > kt // (QW // P) and state["q_sp_done"] > qw:
                    emit_scores(w_idx, kt)
                    sc_i += 1
                    progress += 1
            # 3) filler
            progress += drain_filler(4)
            if progress == 0:
                # stuck: force whatever is forceable (pv without margin, then scores)
                if pv_i < NPV and pv_i // 2 < sc_i and state["v_done"] > pv_list[pv_i][1]:
                    w_idx, kt, h2 = pv_list[pv_i]
                    emit_pv(w_idx, kt, h2)
                    if kt == SM - 1:
                        emit_norm_h2(w_idx, h2)
                    pv_i += 1
                elif sc_i < NSCP:
                    w_idx, kt = sc_list[sc_i]
                    emit_scores(w_idx, kt)
                    sc_i += 1
                else:
                    raise RuntimeError("scheduler deadlock")


_NC_CACHE = None


def _get_nc():
    global _NC_CACHE
    if _NC_CACHE is None:
        _NC_CACHE = _build_kernel()
    return _NC_CACHE


def _make_in_maps(query, key, value, W1, b1, W2, b2, W3, b3, W4, b4):
    in_maps = []
    for c in range(N_CORES):
        b, g = divmod(c, 4)
        gs = slice(g * F, (g + 1) * F)
        in_maps.append({
            "xq_t": np.ascontiguousarray(query[b].T).astype(BF16),
            "xk_t": np.ascontiguousarray(key[b].T).astype(BF16),
            "xv_t": np.ascontiguousarray(value[b].T).astype(BF16),
            "w1t": np.ascontiguousarray(W1[gs, :].T).astype(BF16),
            "w2t": np.ascontiguousarray(W2[gs, :].T).astype(BF16),
            "w3t": np.ascontiguousarray(W3[gs, :].T).astype(BF16),
            "w4t": np.ascontiguousarray(W4[:, gs].T).astype(BF16),
            "b1c": np.ascontiguousarray(b1[gs].reshape(F // P, P).T).astype(np.float32),
            "b2c": np.ascontiguousarray(b2[gs].reshape(F // P, P).T).astype(np.float32),
            "b3r": b3[gs].reshape(1, F).astype(BF16),
        })
    return in_maps


def kernel(query, key, value, W1, b1, W2, b2, W3, b3, W4, b4, _trace=False, _tmpdir=None):
    args = [np.asarray(a) for a in (query, key, value, W1, b1, W2, b2, W3, b3, W4, b4)]
    nc = _get_nc()
    in_maps = _make_in_maps(*args)
    res = run_bass_kernel_spmd(
        nc, in_maps, core_ids=list(range(N_CORES)),
        trace=_trace, tmpdir=_tmpdir,
    )
    b4_f = args[10].astype(np.float32)
    full = np.zeros((B, S, D), np.float32)
    for c in range(N_CORES):
        full[c // 4] += np.asarray(res.results[c]["out"], dtype=np.float32)
    full += b4_f[None, None, :]
    kernel.last_results = res
    return full
